# revision 45
# baseline (speedup 1.0000x reference)
"""Trainium2 Bass kernel for LoFTR-style linear attention (nn_MultiHeadAttention).

Math (per batch b, per head h of 8, head dim 32, E=256, L=8192):
  Q = q @ Wq.T + bq ; K = k @ Wk.T + bk ; V = v @ Wv.T + bv
  Qf = elu(Q)+1 ; Kf = elu(K)+1
  KV_h = Kf_h.T @ (V_h/L) ; Ksum_h = sum_s Kf_h
  Z = 1/(Qf_h . Ksum_h + eps)
  msg_h = (Qf_h @ KV_h) * Z * L
  out = msg @ Wm.T

Kernel strategy (one core per batch, 8 cores):
  - All matmuls in bf16 (PE 1 cyc/row) with fp32 PSUM accumulation.
  - The /L and *L cancel exactly; eps is negligible (Zi ~ 1e4) and dropped.
  - elu(x)+1 == min(exp(x),1) + relu(x).
  - V projection is deferred via associativity:
      KV = Kf.T @ (v @ Wv.T + bv) = (Kf.T @ v_raw) @ Wv.T + Ksum x bv
    so phase A accumulates KV_raw = Kf.T @ [v_raw | 1] (raw v as matmul rhs,
    no V transpose, no V projection) and the 256x256 Wv projection + bias
    fold happen once at the phase boundary (PE-transpose of KV_raw + 4 small
    matmuls).
  - K projects in NATURAL layout (lhsT = kT blocks, W streams) so Kf comes
    out ready to be the KV lhsT; bk is injected into the projection PSUM by
    a K=1 ones-outer-product matmul (no elementwise bias add).
  - Q projects in T layout (W stationary, qT streams) during PHASE A (it has
    no KV dependency); Qf stays resident in SBUF so phase B is only
    Zi -> 1/Zi -> expand -> msg -> out. bq rides the ACT per-partition bias.
  - Z: Zi for all 8 heads in one [8,512] PSUM tile via masked-Ksum lhsT,
    1/Zi = exp(-ln(Zi)) on ACT (bf16 out, dodges the slow DVE reciprocal),
    expanded to [128,512] with a 0/1 matmul, multiplied into Qf.
  - Inputs are cast fp32->bf16 during the SWDGE (gpsimd) DMA load; q/k are
    transposed ON THE PE (4 blocks batched per PSUM bank + one copy out).
    No DMA xbar transposes anywhere: the Tile scheduler serializes those
    against ALL in-flight DMAs (HW deadlock guard), which cost ~10us per
    transpose in pipeline stalls. With PE transposes the DMA system carries
    only HBM loads/stores and pipelines freely.
  - K feature-map elementwise ops run on [128,512] pairs; output stores are
    batched per 512 rows.
"""

import sys

for p in ("/opt/trn_rl_repo", "/opt/trn_rl_repo/concourse"):
    if p not in sys.path:
        sys.path.insert(0, p)

from contextlib import ExitStack

import ml_dtypes
import numpy as np

import concourse.bass as bass
import concourse.tile as tile
from concourse import mybir
from concourse.bass_utils import run_bass_kernel_spmd

F32 = mybir.dt.float32
BF16 = mybir.dt.bfloat16
AF = mybir.ActivationFunctionType
OP = mybir.AluOpType

B, L, E = 8, 8192, 256
H, D = 8, 32
NCORES = 8

LBLK = 2048           # rows per cast-load / input-transpose batch
NBLK = L // LBLK      # 4
TPB = LBLK // 128     # 128-row tiles per block = 16
GRP = 512             # rows per phase-B group
NGRP = L // GRP       # 16
GPB = LBLK // GRP     # groups per block = 4
TPG = GRP // 128      # tiles per group = 4

# The xbar transpose instruction needs a 3D non-mergeable out AP (pad stride
# 132) but the HW packs the transposed 128x128 blocks contiguously at stride
# 128 — so allocate flat tiles, hand the instruction a fake-padded AP, and
# read results back at contiguous offsets (verified by probe on HW).
XSTRIDE = 132


def build_nc(fix_waits=True, xs=128):
    """xs: stride at which transposed 128-col blocks are read back from the
    xbar output tile. The HW packs them at 128 regardless of the fake padded
    out AP; CoreSim honors the 132 stride — pass xs=132 to validate there."""
    nc = bass.Bass()

    q_h = nc.declare_dram_parameter("q", [L, E], F32, isOutput=False)
    k_h = nc.declare_dram_parameter("k", [L, E], F32, isOutput=False)
    v_h = nc.declare_dram_parameter("v", [L, E], F32, isOutput=False)
    wq_h = nc.declare_dram_parameter("wqT", [E, E], BF16, isOutput=False)
    wk_h = nc.declare_dram_parameter("wkT", [E, E], BF16, isOutput=False)
    wv_h = nc.declare_dram_parameter("wvT", [E, E], BF16, isOutput=False)
    wm_h = nc.declare_dram_parameter("wmT", [E, E], BF16, isOutput=False)
    bq_h = nc.declare_dram_parameter("bq2", [128, 2], F32, isOutput=False)
    bk2_h = nc.declare_dram_parameter("bk2", [1, 2 * E], BF16, isOutput=False)
    ones1_h = nc.declare_dram_parameter("ones1", [1, 128], BF16, isOutput=False)
    bvb_h = nc.declare_dram_parameter("bvb", [128, E], F32, isOutput=False)
    mbd_h = nc.declare_dram_parameter("maskbd", [128, 128], F32, isOutput=False)
    mh8_h = nc.declare_dram_parameter("mh8", [128, 2, 8], F32, isOutput=False)
    em8_h = nc.declare_dram_parameter("em8", [8, 2, 128], BF16, isOutput=False)
    id_h = nc.declare_dram_parameter("ident", [128, 128], BF16, isOutput=False)
    out_h = nc.declare_dram_parameter("out", [L, E], F32, isOutput=True)

    with ExitStack() as ctx:
        tc = ctx.enter_context(tile.TileContext(nc))

        const = ctx.enter_context(tc.tile_pool(name="const", bufs=1))
        knatp = ctx.enter_context(tc.tile_pool(name="knat", bufs=3))
        ktp = ctx.enter_context(tc.tile_pool(name="kt", bufs=3))
        vxp = ctx.enter_context(tc.tile_pool(name="vx", bufs=2))
        erp = ctx.enter_context(tc.tile_pool(name="er", bufs=4))
        kfnp = ctx.enter_context(tc.tile_pool(name="kfn", bufs=4))
        bndp = ctx.enter_context(tc.tile_pool(name="bnd", bufs=1))

        ctx_kv = ctx.enter_context(ExitStack())
        ps_kv = ctx_kv.enter_context(tc.tile_pool(name="ps_kv", bufs=1, space="PSUM"))

        # ---- constants -------------------------------------------------
        def load_w(h, tag):
            t = const.tile([128, 2, E], BF16, tag=tag)
            nc.sync.dma_start(t[:], h[:].rearrange("(c p) e -> p c e", p=128))
            return t

        wq = load_w(wq_h, "wq")
        wk = load_w(wk_h, "wk")
        wv = load_w(wv_h, "wv")
        wm = load_w(wm_h, "wm")
        bq = const.tile([128, 2], F32)
        nc.sync.dma_start(bq[:], bq_h[:])
        bk2 = const.tile([1, 2 * E], BF16)
        nc.sync.dma_start(bk2[:], bk2_h[:])
        ones1 = const.tile([1, 128], BF16)
        nc.sync.dma_start(ones1[:], ones1_h[:])
        bvb = const.tile([128, E], F32)
        nc.sync.dma_start(bvb[:], bvb_h[:])
        mbd = const.tile([128, 128], F32)
        nc.sync.dma_start(mbd[:], mbd_h[:])
        mh8 = const.tile([128, 2, 8], F32)
        nc.sync.dma_start(mh8[:], mh8_h[:])
        em8 = const.tile([8, 2, 128], BF16)
        nc.sync.dma_start(em8[:], em8_h[:])
        ident = const.tile([128, 128], BF16)
        nc.sync.dma_start(ident[:], id_h[:])

        # persistent KV_raw accumulators: kvp[c] = Kf[:, c-chunk].T @ [v | 1]
        kv0 = ps_kv.tile([128, 257], F32, tag="kv0")
        kv1 = ps_kv.tile([128, 257], F32, tag="kv1")
        kvp = (kv0, kv1)

        def cast_load(src_h, l0, tag):
            """fp32 HBM [LBLK,256] -> bf16 SBUF [128, 16, 256] (l on part)."""
            t = knatp.tile([128, TPB, E], BF16, tag=tag)
            nc.gpsimd.dma_start(
                t[:],
                src_h[l0 : l0 + LBLK, :].rearrange("(b p) e -> p b e", p=128),
            )
            return t

        # ===== phase A: K,v_raw -> KV_raw accumulation; Q -> Qf resident =
        # All input transposes are done on the PE (4 transposes batched into
        # one PSUM bank, one copy out) — the DMA system never runs an xbar
        # transpose, so loads/stores pipeline freely.
        qfull = [
            const.tile([128, NGRP, GRP], BF16, tag=f"qfull{ec}", name=f"qfull{ec}")
            for ec in (0, 1)
        ]
        ctx_a = ctx.enter_context(ExitStack())
        ps_k = ctx_a.enter_context(tc.tile_pool(name="ps_k", bufs=2, space="PSUM"))
        ps_qt = ctx_a.enter_context(tc.tile_pool(name="ps_qt", bufs=2, space="PSUM"))
        ps_xp = ctx_a.enter_context(tc.tile_pool(name="ps_xp", bufs=2, space="PSUM"))

        def pe_T(nat_t, cc, half, tag, on_act):
            """transpose 8 natural 128x128 blocks (l-tiles 8h..8h+7 of e-chunk
            cc) into one SBUF tile [128 e, 8*128 l] (covers 2 groups)."""
            xp = ps_xp.tile([128, 2 * TPG, 128], BF16, tag="xp")
            for j in range(2 * TPG):
                t = half * 2 * TPG + j
                nc.tensor.transpose(
                    xp[:, j, :], nat_t[:, t, cc * 128 : (cc + 1) * 128], ident[:]
                )
            sb = ktp.tile([128, 2 * TPG * 128], BF16, tag=tag)
            if on_act:
                nc.scalar.copy(sb[:], xp[:].rearrange("p a b -> p (a b)"))
            else:
                nc.vector.tensor_copy(sb[:], xp[:].rearrange("p a b -> p (a b)"))
            return sb

        for blk in range(NBLK):
            l0 = blk * LBLK
            kn = cast_load(k_h, l0, "kn")
            qn = cast_load(q_h, l0, "qn")
            # raw v rows, bf16, with a ones column at 256
            vx = vxp.tile([128, TPB, 258], BF16, tag="vx")
            nc.gpsimd.dma_start(
                vx[:, :, 0:256],
                v_h[l0 : l0 + LBLK, :].rearrange("(b p) e -> p b e", p=128),
            )
            nc.gpsimd.memset(vx[:, :, 256:257], 1.0)

            for gi in range(GPB):
                g = blk * GPB + gi
                if gi % 2 == 0:
                    kTh = [
                        pe_T(kn, cc, gi // 2, f"kT{cc}", on_act=(cc == 0))
                        for cc in (0, 1)
                    ]
                    qTh = [
                        pe_T(qn, cc, gi // 2, f"qT{cc}", on_act=(cc == 0))
                        for cc in (0, 1)
                    ]
                goff = (gi % 2) * GRP
                kT = [kTh[cc][:, goff : goff + GRP] for cc in (0, 1)]
                qT = [qTh[cc][:, goff : goff + GRP] for cc in (0, 1)]

                # Q T-layout projection + feature map into the resident tile
                for ec in (0, 1):
                    esl = slice(ec * 128, (ec + 1) * 128)
                    qt_ps = ps_qt.tile([128, GRP], F32, tag="qt")
                    nc.tensor.matmul(
                        qt_ps[:], wq[:, 0, esl], qT[0][:], start=True, stop=False
                    )
                    nc.tensor.matmul(
                        qt_ps[:], wq[:, 1, esl], qT[1][:], start=False, stop=True
                    )
                    qe_t = erp.tile([128, GRP], BF16, tag="qe")
                    nc.scalar.activation(
                        qe_t[:], qt_ps[:], AF.Exp, bias=bq[:, ec : ec + 1]
                    )
                    qr_t = erp.tile([128, GRP], BF16, tag="qr")
                    nc.scalar.activation(
                        qr_t[:], qt_ps[:], AF.Relu, bias=bq[:, ec : ec + 1]
                    )
                    nc.vector.scalar_tensor_tensor(
                        qfull[ec][:, g, :], qe_t[:], 1.0, qr_t[:], OP.min, OP.add
                    )

                # K: two l-tiles paired per PSUM bank so the feature-map
                # elementwise ops run at [128,512] granularity
                for pr in (0, 1):
                    t0 = gi * TPG + pr * 2
                    k_ps = ps_k.tile([128, 2, E], F32, tag="k")
                    # bk bias for both halves via a K=1 outer-product matmul,
                    # first in the accumulation group (clears the region)
                    nc.tensor.matmul(
                        k_ps[:].rearrange("p a b -> p (a b)"), ones1[:], bk2[:],
                        start=True, stop=False, skip_group_check=True,
                    )
                    for tt in (0, 1):
                        lsl = slice((pr * 2 + tt) * 128, (pr * 2 + tt + 1) * 128)
                        nc.tensor.matmul(
                            k_ps[:, tt, :], kT[0][:, lsl], wk[:, 0, :],
                            start=False, stop=False, skip_group_check=True,
                        )
                        nc.tensor.matmul(
                            k_ps[:, tt, :], kT[1][:, lsl], wk[:, 1, :],
                            start=False, stop=True, skip_group_check=True,
                        )
                    # f = min(exp(x+bk),1) + relu(x+bk), bk along free dim
                    kbf = k_ps[:].rearrange("p a b -> p (a b)")
                    e_t = erp.tile([128, 2 * E], BF16, tag="e")
                    nc.scalar.activation(e_t[:], kbf, AF.Exp)
                    r_t = erp.tile([128, 2 * E], BF16, tag="r")
                    nc.vector.tensor_scalar(r_t[:], kbf, 0.0, None, OP.max)
                    kfn = kfnp.tile([128, 2, E], BF16, tag="kfn")
                    nc.vector.scalar_tensor_tensor(
                        kfn[:].rearrange("p a b -> p (a b)"),
                        e_t[:], 1.0, r_t[:], OP.min, OP.add,
                    )
                    # KV_raw accumulation
                    for tt in (0, 1):
                        ti = blk * TPB + t0 + tt
                        first = ti == 0
                        last = ti == L // 128 - 1
                        for c in (0, 1):
                            nc.tensor.matmul(
                                kvp[c][:],
                                kfn[:, tt, c * 128 : (c + 1) * 128],
                                vx[:, t0 + tt, 0:257],
                                start=first,
                                stop=last,
                            )

        ctx_a.close()

        # ====== phase boundary: KV_true = KV_raw @ Wv.T + Ksum x bv =====
        ksum_sb = []
        kvr_sb = []
        for c in (0, 1):
            ks = bndp.tile([128, 1], F32, tag=f"ksum{c}")
            nc.vector.tensor_copy(ks[:], kvp[c][:, 256:257])
            ksum_sb.append(ks)
            kr = bndp.tile([128, 256], BF16, tag=f"kvr{c}")
            nc.scalar.copy(kr[:], kvp[c][:, 0:256])
            kvr_sb.append(kr)

        ctx_kv.close()

        ctx_b = ctx.enter_context(ExitStack())
        ps_b = ctx_b.enter_context(tc.tile_pool(name="ps_b", bufs=2, space="PSUM"))
        # transpose KV_raw (256x256) via PE: kvrT[b][e_loc, d_glob]
        kvrT = [
            bndp.tile([128, 256], BF16, tag=f"kvrT{b}", name=f"kvrT{b}")
            for b in (0, 1)
        ]
        for b in (0, 1):
            for c in (0, 1):
                t_ps = ps_b.tile([128, 128], F32, tag="tp")
                nc.tensor.matmul(
                    t_ps[:],
                    kvr_sb[c][:, b * 128 : (b + 1) * 128],
                    ident[:],
                    start=True,
                    stop=True,
                )
                nc.scalar.copy(kvrT[b][:, c * 128 : (c + 1) * 128], t_ps[:])

        # kvt[c][d_loc, v'] = sum_e kvrT[e, d] wv[e, v']  (d in chunk c)
        kvbd = []
        ksbd8 = []
        for c in (0, 1):
            csl = slice(c * 128, (c + 1) * 128)
            kvt_ps = ps_b.tile([128, E], F32, tag="kvt")
            nc.tensor.matmul(
                kvt_ps[:], kvrT[0][:, csl], wv[:, 0, :], start=True, stop=False
            )
            nc.tensor.matmul(
                kvt_ps[:], kvrT[1][:, csl], wv[:, 1, :], start=False, stop=True
            )
            # block-diagonal extract + bv fold (only e' in chunk c survives)
            tmp = bndp.tile([128, 128], F32, tag=f"tmp{c}")
            nc.vector.tensor_scalar(
                tmp[:], bvb[:, csl], ksum_sb[c][:], None, OP.mult
            )
            s_t = bndp.tile([128, 128], F32, tag=f"sum{c}")
            nc.vector.tensor_tensor(s_t[:], kvt_ps[:, csl], tmp[:], OP.add)
            kv_t = bndp.tile([128, 128], BF16, tag=f"kvbd{c}")
            nc.vector.tensor_tensor(kv_t[:], s_t[:], mbd[:], OP.mult)
            kvbd.append(kv_t)
            ks_t = bndp.tile([128, 8], BF16, tag=f"ksbd{c}")
            nc.vector.tensor_scalar(
                ks_t[:], mh8[:, c, :], ksum_sb[c][:], None, OP.mult
            )
            ksbd8.append(ks_t)

        ctx_b.close()

        # ================= phase B: Z -> msg -> out =====================
        zp = ctx.enter_context(tc.tile_pool(name="z", bufs=2))
        msp = ctx.enter_context(tc.tile_pool(name="msgts", bufs=3))
        outp = ctx.enter_context(tc.tile_pool(name="outsb", bufs=3))
        ps_zi = ctx.enter_context(tc.tile_pool(name="ps_zi", bufs=2, space="PSUM"))
        ps_ze = ctx.enter_context(tc.tile_pool(name="ps_ze", bufs=2, space="PSUM"))
        ps_mt = ctx.enter_context(tc.tile_pool(name="ps_mt", bufs=2, space="PSUM"))
        ps_o = ctx.enter_context(tc.tile_pool(name="ps_o", bufs=2, space="PSUM"))
        for g in range(NGRP):
            qft = [qfull[ec][:, g, :] for ec in (0, 1)]

            # Zi for all 8 heads: [8, 512]
            zi_ps = ps_zi.tile([8, GRP], F32, tag="zi")
            nc.tensor.matmul(zi_ps[:], ksbd8[0][:], qft[0], start=True, stop=False)
            nc.tensor.matmul(zi_ps[:], ksbd8[1][:], qft[1], start=False, stop=True)
            # 1/Zi = exp(-ln(Zi)) on ACT, bf16 out
            zln = zp.tile([8, GRP], F32, tag="zln")
            nc.scalar.activation(zln[:], zi_ps[:], AF.Ln)
            zrb = zp.tile([8, GRP], BF16, tag="zrb")
            nc.scalar.activation(zrb[:], zln[:], AF.Exp, scale=-1.0)

            mts = []
            for c in (0, 1):
                ze_ps = ps_ze.tile([128, GRP], F32, tag="ze")
                nc.tensor.matmul(
                    ze_ps[:], em8[:, c, :], zrb[:], start=True, stop=True
                )
                qfts = msp.tile([128, GRP], BF16, tag=f"qfts{c}")
                nc.vector.tensor_tensor(qfts[:], qft[c], ze_ps[:], OP.mult)
                mt_ps = ps_mt.tile([128, GRP], F32, tag="mt")
                nc.tensor.matmul(
                    mt_ps[:], kvbd[c][:], qfts[:], start=True, stop=True
                )
                m_t = msp.tile([128, GRP], BF16, tag=f"mts{c}")
                if c == 0:
                    nc.vector.tensor_copy(m_t[:], mt_ps[:])
                else:
                    nc.scalar.copy(m_t[:], mt_ps[:])
                mts.append(m_t)

            o_sb = outp.tile([128, TPG, E], F32, tag="osb")
            for th in (0, 1):
                o_ps = ps_o.tile([128, 2, E], F32, tag="o")
                for tt in (0, 1):
                    t = th * 2 + tt
                    lsl = slice(t * 128, (t + 1) * 128)
                    nc.tensor.matmul(
                        o_ps[:, tt, :], mts[0][:, lsl], wm[:, 0, :],
                        start=True, stop=False,
                    )
                    nc.tensor.matmul(
                        o_ps[:, tt, :], mts[1][:, lsl], wm[:, 1, :],
                        start=False, stop=True,
                    )
                if th == 0:
                    nc.scalar.copy(o_sb[:, 0:2, :], o_ps[:])
                else:
                    nc.vector.tensor_copy(o_sb[:, 2:4, :], o_ps[:])
            nc.sync.dma_start(
                out_h[g * GRP : (g + 1) * GRP, :].rearrange(
                    "(t p) e -> p t e", p=128
                ),
                o_sb[:],
            )

    if fix_waits:
        _fix_xpose_waits(nc)
    return nc


_WAIT_EXEMPT = {"InstEventSemaphore", "InstUnconditionalBranch", "InstISA"}


def _fix_xpose_waits(nc):
    """Several TPB ISA structs hold at most 2 sem-wait slots (the xpose DMA
    even fewer), but the Tile scheduler can emit more (e.g. its conservative
    xbar serialization waits on every in-flight DMA lane). Move excess waits
    onto sequencer EventSemaphore instructions inserted immediately before
    the instruction on the same engine — program order keeps semantics."""
    n = 0
    for fn in nc.m.functions:
        for blk in fn.blocks:
            il = blk.instructions
            new = []
            changed = False
            for inst in il:
                tname = type(inst).__name__
                if tname not in _WAIT_EXEMPT:
                    limit = 0 if tname == "InstDmaTransposeAnt" else 1
                    si = inst.sync_info
                    waits = list(si.on_wait) if si is not None and si.on_wait else []
                    if len(waits) > limit:
                        move, keep = waits[: len(waits) - limit], waits[len(waits) - limit :]
                        for w in move:
                            es = mybir.InstEventSemaphore(
                                name=f"wait_fence_{n}",
                                ins=[],
                                outs=[],
                                engine=inst.engine,
                            )
                            es.sync_info = mybir.SyncInfo(on_wait=[w], on_update=[])
                            new.append(es)
                            n += 1
                        inst.sync_info = mybir.SyncInfo(
                            on_wait=keep,
                            on_update=list(si.on_update) if si.on_update else [],
                        )
                        changed = True
                new.append(inst)
            if changed:
                blk.instructions = new


_NC = None


def _get_nc():
    global _NC
    if _NC is None:
        _NC = build_nc()
    return _NC


def _host_consts(inputs):
    bf = ml_dtypes.bfloat16
    Wq, Wk, Wv, Wm = (np.asarray(inputs[n], np.float32) for n in ("Wq", "Wk", "Wv", "Wm"))
    bq, bk, bv = (np.asarray(inputs[n], np.float32) for n in ("bq", "bk", "bv"))

    consts = {
        "wqT": np.ascontiguousarray(Wq.T).astype(bf),
        "wkT": np.ascontiguousarray(Wk.T).astype(bf),
        "wvT": np.ascontiguousarray(Wv.T).astype(bf),
        "wmT": np.ascontiguousarray(Wm.T).astype(bf),
        "bq2": np.ascontiguousarray(bq.reshape(2, 128).T),
        "bk2": np.ascontiguousarray(np.tile(bk, 2)[None, :]).astype(bf),
        "ones1": np.ones((1, 128), np.float32).astype(bf),
        "bvb": np.ascontiguousarray(np.broadcast_to(bv, (128, E))),
    }
    p = np.arange(128)
    consts["maskbd"] = ((p[:, None] // 32) == (p[None, :] // 32)).astype(np.float32)
    # mh8[p, c, j] = 1 iff head j == 4c + p//32
    mh8 = np.zeros((128, 2, 8), np.float32)
    for c in (0, 1):
        mh8[:, c, :] = (np.arange(8)[None, :] == (4 * c + p[:, None] // 32))
    consts["mh8"] = mh8
    em8 = np.zeros((8, 2, 128), np.float32)
    for c in (0, 1):
        em8[:, c, :] = (np.arange(8)[:, None] == (4 * c + p[None, :] // 32))
    consts["em8"] = em8.astype(bf)
    consts["ident"] = np.eye(128, dtype=np.float32).astype(bf)
    return consts


def _make_in_maps(inputs):
    consts = _host_consts(inputs)
    q = np.asarray(inputs["q"], np.float32)
    k = np.asarray(inputs["k"], np.float32)
    v = np.asarray(inputs["v"], np.float32)

    in_maps = []
    for b in range(NCORES):
        m = dict(consts)
        m["q"] = np.ascontiguousarray(q[b])
        m["k"] = np.ascontiguousarray(k[b])
        m["v"] = np.ascontiguousarray(v[b])
        in_maps.append(m)
    return in_maps


def kernel(**inputs):
    nc = _get_nc()
    res = run_bass_kernel_spmd(nc, _make_in_maps(inputs), list(range(NCORES)))
    out = np.stack([np.asarray(res.results[b]["out"]) for b in range(NCORES)])
    return out.astype(np.float32)


def kernel_traced(**inputs):
    """Like kernel() but with NTFF profiling; returns (out, BassKernelResults)."""
    nc = _get_nc()
    res = run_bass_kernel_spmd(
        nc, _make_in_maps(inputs), list(range(NCORES)), trace=True
    )
    out = np.stack([np.asarray(res.results[b]["out"]) for b in range(NCORES)])
    return out.astype(np.float32), res


if __name__ == "__main__":
    rng = np.random.default_rng(0)
    ins = {
        "q": rng.standard_normal((B, L, E), np.float32),
        "k": rng.standard_normal((B, L, E), np.float32),
        "v": rng.standard_normal((B, L, E), np.float32),
        "Wq": rng.standard_normal((E, E), np.float32) / 16,
        "bq": rng.standard_normal(E).astype(np.float32) * 0.01,
        "Wk": rng.standard_normal((E, E), np.float32) / 16,
        "bk": rng.standard_normal(E).astype(np.float32) * 0.01,
        "Wv": rng.standard_normal((E, E), np.float32) / 16,
        "bv": rng.standard_normal(E).astype(np.float32) * 0.01,
        "Wm": rng.standard_normal((E, E), np.float32) / 16,
    }
    out = kernel(**ins)
    print("out", out.shape, out.dtype, np.abs(out).mean())


# revision 47
# speedup vs baseline: 1.0555x; 1.0555x over previous
"""Trainium2 Bass kernel for LoFTR-style linear attention (nn_MultiHeadAttention).

Math (per batch b, per head h of 8, head dim 32, E=256, L=8192):
  Q = q @ Wq.T + bq ; K = k @ Wk.T + bk ; V = v @ Wv.T + bv
  Qf = elu(Q)+1 ; Kf = elu(K)+1
  KV_h = Kf_h.T @ (V_h/L) ; Ksum_h = sum_s Kf_h
  Z = 1/(Qf_h . Ksum_h + eps)
  msg_h = (Qf_h @ KV_h) * Z * L
  out = msg @ Wm.T

Kernel strategy (one core per batch, 8 cores):
  - All matmuls in bf16 (PE 1 cyc/row) with fp32 PSUM accumulation.
  - The /L and *L cancel exactly; eps is negligible (Zi ~ 1e4) and dropped.
  - elu(x)+1 == min(exp(x),1) + relu(x).
  - V projection is deferred via associativity:
      KV = Kf.T @ (v @ Wv.T + bv) = (Kf.T @ v_raw) @ Wv.T + Ksum x bv
    so phase A accumulates KV_raw = Kf.T @ [v_raw | 1] (raw v as matmul rhs,
    no V transpose, no V projection) and the 256x256 Wv projection + bias
    fold happen once at the phase boundary (PE-transpose of KV_raw + 4 small
    matmuls).
  - K projects in NATURAL layout (lhsT = kT blocks, W streams) so Kf comes
    out ready to be the KV lhsT; bk is added with a broadcast bias tile.
  - Q projects in T layout (W stationary, qT streams); bq rides the ACT
    per-partition bias. Z: Zi for all 8 heads in one [8,512] PSUM tile via
    masked-Ksum lhsT, 1/Zi = exp(-ln(Zi)) on ACT (bf16 out), expanded to
    [128,512] with a 0/1 matmul and fused into msg via one tensor_tensor.
  - Inputs are cast fp32->bf16 during the SWDGE (gpsimd) DMA load; q/k are
    xbar-transposed (2-byte dtype) to put the contraction dim on partitions.
  - Output stores are batched per 512 rows.
"""

import sys

for p in ("/opt/trn_rl_repo", "/opt/trn_rl_repo/concourse"):
    if p not in sys.path:
        sys.path.insert(0, p)

from contextlib import ExitStack

import ml_dtypes
import numpy as np

import concourse.bass as bass
import concourse.tile as tile
from concourse import mybir
from concourse.bass_utils import run_bass_kernel_spmd

F32 = mybir.dt.float32
BF16 = mybir.dt.bfloat16
AF = mybir.ActivationFunctionType
OP = mybir.AluOpType

B, L, E = 8, 8192, 256
H, D = 8, 32
NCORES = 8

LBLK = 2048           # rows per cast-load / input-transpose batch
NBLK = L // LBLK      # 4
TPB = LBLK // 128     # 128-row tiles per block = 16
GRP = 512             # rows per phase-B group
NGRP = L // GRP       # 16
GPB = LBLK // GRP     # groups per block = 4
TPG = GRP // 128      # tiles per group = 4

# The xbar transpose instruction needs a 3D non-mergeable out AP (pad stride
# 132) but the HW packs the transposed 128x128 blocks contiguously at stride
# 128 — so allocate flat tiles, hand the instruction a fake-padded AP, and
# read results back at contiguous offsets (verified by probe on HW).
XSTRIDE = 132


def build_nc(fix_waits=True, xs=128):
    """xs: stride at which transposed 128-col blocks are read back from the
    xbar output tile. The HW packs them at 128 regardless of the fake padded
    out AP; CoreSim honors the 132 stride — pass xs=132 to validate there."""
    nc = bass.Bass()

    q_h = nc.declare_dram_parameter("q", [L, E], F32, isOutput=False)
    k_h = nc.declare_dram_parameter("k", [L, E], F32, isOutput=False)
    v_h = nc.declare_dram_parameter("v", [L, E], F32, isOutput=False)
    wq_h = nc.declare_dram_parameter("wqT", [E, E], BF16, isOutput=False)
    wk_h = nc.declare_dram_parameter("wkT", [E, E], BF16, isOutput=False)
    wv_h = nc.declare_dram_parameter("wvT", [E, E], BF16, isOutput=False)
    wm_h = nc.declare_dram_parameter("wmT", [E, E], BF16, isOutput=False)
    bq_h = nc.declare_dram_parameter("bq2", [128, 2], F32, isOutput=False)
    bk2_h = nc.declare_dram_parameter("bk2", [1, 2 * E], BF16, isOutput=False)
    ones1_h = nc.declare_dram_parameter("ones1", [1, 128], BF16, isOutput=False)
    bvb_h = nc.declare_dram_parameter("bvb", [128, E], F32, isOutput=False)
    mbd_h = nc.declare_dram_parameter("maskbd", [128, 128], F32, isOutput=False)
    mh8_h = nc.declare_dram_parameter("mh8", [128, 2, 8], F32, isOutput=False)
    em8_h = nc.declare_dram_parameter("em8", [8, 2, 128], BF16, isOutput=False)
    id_h = nc.declare_dram_parameter("ident", [128, 128], BF16, isOutput=False)
    out_h = nc.declare_dram_parameter("out", [L, E], F32, isOutput=True)

    with ExitStack() as ctx:
        tc = ctx.enter_context(tile.TileContext(nc))

        const = ctx.enter_context(tc.tile_pool(name="const", bufs=1))
        knatp = ctx.enter_context(tc.tile_pool(name="knat", bufs=3))
        ktp = ctx.enter_context(tc.tile_pool(name="kt", bufs=3))
        vxp = ctx.enter_context(tc.tile_pool(name="vx", bufs=3))
        erp = ctx.enter_context(tc.tile_pool(name="er", bufs=4))
        kfnp = ctx.enter_context(tc.tile_pool(name="kfn", bufs=4))
        bndp = ctx.enter_context(tc.tile_pool(name="bnd", bufs=1))

        ctx_kv = ctx.enter_context(ExitStack())
        ps_kv = ctx_kv.enter_context(tc.tile_pool(name="ps_kv", bufs=1, space="PSUM"))

        # ---- constants -------------------------------------------------
        def load_w(h, tag):
            t = const.tile([128, 2, E], BF16, tag=tag)
            nc.sync.dma_start(t[:], h[:].rearrange("(c p) e -> p c e", p=128))
            return t

        # phase-A-critical consts first on the sync queue (ident gates
        # every PE transpose); boundary-only consts go via the scalar
        # HWDGE queue so they don't delay the first block
        ident = const.tile([128, 128], BF16)
        nc.sync.dma_start(ident[:], id_h[:])
        wk = load_w(wk_h, "wk")
        wq = load_w(wq_h, "wq")
        bq = const.tile([128, 2], F32)
        nc.sync.dma_start(bq[:], bq_h[:])
        bk2 = const.tile([1, 2 * E], BF16)
        nc.sync.dma_start(bk2[:], bk2_h[:])
        ones1 = const.tile([1, 128], BF16)
        nc.sync.dma_start(ones1[:], ones1_h[:])
        wv = const.tile([128, 2, E], BF16, tag="wv")
        nc.scalar.dma_start(wv[:], wv_h[:].rearrange("(c p) e -> p c e", p=128))
        wm = const.tile([128, 2, E], BF16, tag="wm")
        nc.scalar.dma_start(wm[:], wm_h[:].rearrange("(c p) e -> p c e", p=128))
        bvb = const.tile([128, E], F32)
        nc.scalar.dma_start(bvb[:], bvb_h[:])
        mbd = const.tile([128, 128], F32)
        nc.scalar.dma_start(mbd[:], mbd_h[:])
        mh8 = const.tile([128, 2, 8], F32)
        nc.scalar.dma_start(mh8[:], mh8_h[:])
        em8 = const.tile([8, 2, 128], BF16)
        nc.scalar.dma_start(em8[:], em8_h[:])

        # persistent KV_raw accumulators: kvp[c] = Kf[:, c-chunk].T @ [v | 1]
        kv0 = ps_kv.tile([128, 257], F32, tag="kv0")
        kv1 = ps_kv.tile([128, 257], F32, tag="kv1")
        kvp = (kv0, kv1)

        def cast_load(src_h, l0, tag):
            """fp32 HBM [LBLK,256] -> bf16 SBUF [128, 16, 256] (l on part)."""
            t = knatp.tile([128, TPB, E], BF16, tag=tag)
            nc.gpsimd.dma_start(
                t[:],
                src_h[l0 : l0 + LBLK, :].rearrange("(b p) e -> p b e", p=128),
            )
            return t

        # ===== phase A: K,v_raw -> KV_raw accumulation; Q -> Qf resident =
        # All input transposes are done on the PE (4 transposes batched into
        # one PSUM bank, one copy out) — the DMA system never runs an xbar
        # transpose, so loads/stores pipeline freely.
        qfull = [
            const.tile([128, NGRP, GRP], BF16, tag=f"qfull{ec}", name=f"qfull{ec}")
            for ec in (0, 1)
        ]
        ctx_a = ctx.enter_context(ExitStack())
        ps_k = ctx_a.enter_context(tc.tile_pool(name="ps_k", bufs=2, space="PSUM"))
        ps_qt = ctx_a.enter_context(tc.tile_pool(name="ps_qt", bufs=2, space="PSUM"))
        ps_xp = ctx_a.enter_context(tc.tile_pool(name="ps_xp", bufs=2, space="PSUM"))

        def pe_T(nat_t, cc, quad, tag, on_act):
            """transpose 4 natural 128x128 blocks (l-tiles 4q..4q+3 of e-chunk
            cc) into one SBUF tile [128 e, 4*128 l]."""
            xp = ps_xp.tile([128, TPG, 128], BF16, tag="xp")
            for j in range(TPG):
                t = quad * TPG + j
                nc.tensor.transpose(
                    xp[:, j, :], nat_t[:, t, cc * 128 : (cc + 1) * 128], ident[:]
                )
            sb = ktp.tile([128, TPG * 128], BF16, tag=tag)
            if on_act:
                nc.scalar.copy(sb[:], xp[:].rearrange("p a b -> p (a b)"))
            else:
                nc.vector.tensor_copy(sb[:], xp[:].rearrange("p a b -> p (a b)"))
            return sb

        for blk in range(NBLK):
            l0 = blk * LBLK
            kn = cast_load(k_h, l0, "kn")
            qn = cast_load(q_h, l0, "qn")
            # raw v rows, bf16, with a ones column at 256
            vx = vxp.tile([128, TPB, 258], BF16, tag="vx")
            nc.gpsimd.dma_start(
                vx[:, :, 0:256],
                v_h[l0 : l0 + LBLK, :].rearrange("(b p) e -> p b e", p=128),
            )
            nc.gpsimd.memset(vx[:, :, 256:257], 1.0)

            for gi in range(GPB):
                g = blk * GPB + gi
                kT = [pe_T(kn, cc, gi, f"kT{cc}", on_act=(cc == 0)) for cc in (0, 1)]
                qT = [pe_T(qn, cc, gi, f"qT{cc}", on_act=(cc == 0)) for cc in (0, 1)]

                # Q T-layout projection + feature map into the resident tile
                for ec in (0, 1):
                    esl = slice(ec * 128, (ec + 1) * 128)
                    qt_ps = ps_qt.tile([128, GRP], F32, tag="qt")
                    nc.tensor.matmul(
                        qt_ps[:], wq[:, 0, esl], qT[0][:], start=True, stop=False
                    )
                    nc.tensor.matmul(
                        qt_ps[:], wq[:, 1, esl], qT[1][:], start=False, stop=True
                    )
                    qe_t = erp.tile([128, GRP], BF16, tag="qe")
                    nc.scalar.activation(
                        qe_t[:], qt_ps[:], AF.Exp, bias=bq[:, ec : ec + 1]
                    )
                    qr_t = erp.tile([128, GRP], BF16, tag="qr")
                    nc.scalar.activation(
                        qr_t[:], qt_ps[:], AF.Relu, bias=bq[:, ec : ec + 1]
                    )
                    nc.vector.scalar_tensor_tensor(
                        qfull[ec][:, g, :], qe_t[:], 1.0, qr_t[:], OP.min, OP.add
                    )

                # K: two l-tiles paired per PSUM bank so the feature-map
                # elementwise ops run at [128,512] granularity
                for pr in (0, 1):
                    t0 = gi * TPG + pr * 2
                    k_ps = ps_k.tile([128, 2, E], F32, tag="k")
                    # bk bias for both halves via a K=1 outer-product matmul,
                    # first in the accumulation group (clears the region)
                    nc.tensor.matmul(
                        k_ps[:].rearrange("p a b -> p (a b)"), ones1[:], bk2[:],
                        start=True, stop=False, skip_group_check=True,
                    )
                    for tt in (0, 1):
                        lsl = slice((pr * 2 + tt) * 128, (pr * 2 + tt + 1) * 128)
                        nc.tensor.matmul(
                            k_ps[:, tt, :], kT[0][:, lsl], wk[:, 0, :],
                            start=False, stop=False, skip_group_check=True,
                        )
                        nc.tensor.matmul(
                            k_ps[:, tt, :], kT[1][:, lsl], wk[:, 1, :],
                            start=False, stop=True, skip_group_check=True,
                        )
                    # f = min(exp(x+bk),1) + relu(x+bk), bk along free dim
                    kbf = k_ps[:].rearrange("p a b -> p (a b)")
                    e_t = erp.tile([128, 2 * E], BF16, tag="e")
                    nc.scalar.activation(e_t[:], kbf, AF.Exp)
                    r_t = erp.tile([128, 2 * E], BF16, tag="r")
                    nc.vector.tensor_scalar(r_t[:], kbf, 0.0, None, OP.max)
                    kfn = kfnp.tile([128, 2, E], BF16, tag="kfn")
                    nc.vector.scalar_tensor_tensor(
                        kfn[:].rearrange("p a b -> p (a b)"),
                        e_t[:], 1.0, r_t[:], OP.min, OP.add,
                    )
                    # KV_raw accumulation
                    for tt in (0, 1):
                        ti = blk * TPB + t0 + tt
                        first = ti == 0
                        last = ti == L // 128 - 1
                        for c in (0, 1):
                            nc.tensor.matmul(
                                kvp[c][:],
                                kfn[:, tt, c * 128 : (c + 1) * 128],
                                vx[:, t0 + tt, 0:257],
                                start=first,
                                stop=last,
                            )

        ctx_a.close()

        # ====== phase boundary: KV_true = KV_raw @ Wv.T + Ksum x bv =====
        ksum_sb = []
        kvr_sb = []
        for c in (0, 1):
            ks = bndp.tile([128, 1], F32, tag=f"ksum{c}")
            nc.vector.tensor_copy(ks[:], kvp[c][:, 256:257])
            ksum_sb.append(ks)
            kr = bndp.tile([128, 256], BF16, tag=f"kvr{c}")
            nc.scalar.copy(kr[:], kvp[c][:, 0:256])
            kvr_sb.append(kr)

        ctx_kv.close()

        ctx_b = ctx.enter_context(ExitStack())
        ps_b = ctx_b.enter_context(tc.tile_pool(name="ps_b", bufs=2, space="PSUM"))
        # transpose KV_raw (256x256) via PE: kvrT[b][e_loc, d_glob]
        kvrT = [
            bndp.tile([128, 256], BF16, tag=f"kvrT{b}", name=f"kvrT{b}")
            for b in (0, 1)
        ]
        for b in (0, 1):
            for c in (0, 1):
                t_ps = ps_b.tile([128, 128], F32, tag="tp")
                nc.tensor.matmul(
                    t_ps[:],
                    kvr_sb[c][:, b * 128 : (b + 1) * 128],
                    ident[:],
                    start=True,
                    stop=True,
                )
                nc.scalar.copy(kvrT[b][:, c * 128 : (c + 1) * 128], t_ps[:])

        # kvt[c][d_loc, v'] = sum_e kvrT[e, d] wv[e, v']  (d in chunk c)
        kvbd = []
        ksbd8 = []
        for c in (0, 1):
            csl = slice(c * 128, (c + 1) * 128)
            kvt_ps = ps_b.tile([128, E], F32, tag="kvt")
            nc.tensor.matmul(
                kvt_ps[:], kvrT[0][:, csl], wv[:, 0, :], start=True, stop=False
            )
            nc.tensor.matmul(
                kvt_ps[:], kvrT[1][:, csl], wv[:, 1, :], start=False, stop=True
            )
            # block-diagonal extract + bv fold (only e' in chunk c survives)
            tmp = bndp.tile([128, 128], F32, tag=f"tmp{c}")
            nc.vector.tensor_scalar(
                tmp[:], bvb[:, csl], ksum_sb[c][:], None, OP.mult
            )
            s_t = bndp.tile([128, 128], F32, tag=f"sum{c}")
            nc.vector.tensor_tensor(s_t[:], kvt_ps[:, csl], tmp[:], OP.add)
            kv_t = bndp.tile([128, 128], BF16, tag=f"kvbd{c}")
            nc.vector.tensor_tensor(kv_t[:], s_t[:], mbd[:], OP.mult)
            kvbd.append(kv_t)
            ks_t = bndp.tile([128, 8], BF16, tag=f"ksbd{c}")
            nc.vector.tensor_scalar(
                ks_t[:], mh8[:, c, :], ksum_sb[c][:], None, OP.mult
            )
            ksbd8.append(ks_t)

        ctx_b.close()

        # ================= phase B: Z -> msg -> out =====================
        zp = ctx.enter_context(tc.tile_pool(name="z", bufs=2))
        msp = ctx.enter_context(tc.tile_pool(name="msgts", bufs=3))
        outp = ctx.enter_context(tc.tile_pool(name="outsb", bufs=2))
        ps_zi = ctx.enter_context(tc.tile_pool(name="ps_zi", bufs=2, space="PSUM"))
        ps_ze = ctx.enter_context(tc.tile_pool(name="ps_ze", bufs=2, space="PSUM"))
        ps_mt = ctx.enter_context(tc.tile_pool(name="ps_mt", bufs=2, space="PSUM"))
        ps_o = ctx.enter_context(tc.tile_pool(name="ps_o", bufs=2, space="PSUM"))
        for g in range(NGRP):
            qft = [qfull[ec][:, g, :] for ec in (0, 1)]

            # Zi for all 8 heads: [8, 512]
            zi_ps = ps_zi.tile([8, GRP], F32, tag="zi")
            nc.tensor.matmul(zi_ps[:], ksbd8[0][:], qft[0], start=True, stop=False)
            nc.tensor.matmul(zi_ps[:], ksbd8[1][:], qft[1], start=False, stop=True)
            # 1/Zi = exp(-ln(Zi)) on ACT, bf16 out
            zln = zp.tile([8, GRP], F32, tag="zln")
            nc.scalar.activation(zln[:], zi_ps[:], AF.Ln)
            zrb = zp.tile([8, GRP], BF16, tag="zrb")
            nc.scalar.activation(zrb[:], zln[:], AF.Exp, scale=-1.0)

            mts = []
            for c in (0, 1):
                ze_ps = ps_ze.tile([128, GRP], F32, tag="ze")
                nc.tensor.matmul(
                    ze_ps[:], em8[:, c, :], zrb[:], start=True, stop=True
                )
                qfts = msp.tile([128, GRP], BF16, tag=f"qfts{c}")
                nc.vector.tensor_tensor(qfts[:], qft[c], ze_ps[:], OP.mult)
                mt_ps = ps_mt.tile([128, GRP], F32, tag="mt")
                nc.tensor.matmul(
                    mt_ps[:], kvbd[c][:], qfts[:], start=True, stop=True
                )
                m_t = msp.tile([128, GRP], BF16, tag=f"mts{c}")
                if c == 0:
                    nc.vector.tensor_copy(m_t[:], mt_ps[:])
                else:
                    nc.scalar.copy(m_t[:], mt_ps[:])
                mts.append(m_t)

            o_sb = outp.tile([128, TPG, E], F32, tag="osb")
            for th in (0, 1):
                o_ps = ps_o.tile([128, 2, E], F32, tag="o")
                for tt in (0, 1):
                    t = th * 2 + tt
                    lsl = slice(t * 128, (t + 1) * 128)
                    nc.tensor.matmul(
                        o_ps[:, tt, :], mts[0][:, lsl], wm[:, 0, :],
                        start=True, stop=False,
                    )
                    nc.tensor.matmul(
                        o_ps[:, tt, :], mts[1][:, lsl], wm[:, 1, :],
                        start=False, stop=True,
                    )
                if th == 0:
                    nc.scalar.copy(o_sb[:, 0:2, :], o_ps[:])
                else:
                    nc.vector.tensor_copy(o_sb[:, 2:4, :], o_ps[:])
            nc.sync.dma_start(
                out_h[g * GRP : (g + 1) * GRP, :].rearrange(
                    "(t p) e -> p t e", p=128
                ),
                o_sb[:],
            )

    if fix_waits:
        _fix_xpose_waits(nc)
    return nc


_WAIT_EXEMPT = {"InstEventSemaphore", "InstUnconditionalBranch", "InstISA"}


def _fix_xpose_waits(nc):
    """Several TPB ISA structs hold at most 2 sem-wait slots (the xpose DMA
    even fewer), but the Tile scheduler can emit more (e.g. its conservative
    xbar serialization waits on every in-flight DMA lane). Move excess waits
    onto sequencer EventSemaphore instructions inserted immediately before
    the instruction on the same engine — program order keeps semantics."""
    n = 0
    for fn in nc.m.functions:
        for blk in fn.blocks:
            il = blk.instructions
            new = []
            changed = False
            for inst in il:
                tname = type(inst).__name__
                if tname not in _WAIT_EXEMPT:
                    limit = 0 if tname == "InstDmaTransposeAnt" else 1
                    si = inst.sync_info
                    waits = list(si.on_wait) if si is not None and si.on_wait else []
                    if len(waits) > limit:
                        move, keep = waits[: len(waits) - limit], waits[len(waits) - limit :]
                        for w in move:
                            es = mybir.InstEventSemaphore(
                                name=f"wait_fence_{n}",
                                ins=[],
                                outs=[],
                                engine=inst.engine,
                            )
                            es.sync_info = mybir.SyncInfo(on_wait=[w], on_update=[])
                            new.append(es)
                            n += 1
                        inst.sync_info = mybir.SyncInfo(
                            on_wait=keep,
                            on_update=list(si.on_update) if si.on_update else [],
                        )
                        changed = True
                new.append(inst)
            if changed:
                blk.instructions = new


_NC = None


def _get_nc():
    global _NC
    if _NC is None:
        _NC = build_nc()
    return _NC


def _host_consts(inputs):
    bf = ml_dtypes.bfloat16
    Wq, Wk, Wv, Wm = (np.asarray(inputs[n], np.float32) for n in ("Wq", "Wk", "Wv", "Wm"))
    bq, bk, bv = (np.asarray(inputs[n], np.float32) for n in ("bq", "bk", "bv"))

    consts = {
        "wqT": np.ascontiguousarray(Wq.T).astype(bf),
        "wkT": np.ascontiguousarray(Wk.T).astype(bf),
        "wvT": np.ascontiguousarray(Wv.T).astype(bf),
        "wmT": np.ascontiguousarray(Wm.T).astype(bf),
        "bq2": np.ascontiguousarray(bq.reshape(2, 128).T),
        "bk2": np.ascontiguousarray(np.tile(bk, 2)[None, :]).astype(bf),
        "ones1": np.ones((1, 128), np.float32).astype(bf),
        "bvb": np.ascontiguousarray(np.broadcast_to(bv, (128, E))),
    }
    p = np.arange(128)
    consts["maskbd"] = ((p[:, None] // 32) == (p[None, :] // 32)).astype(np.float32)
    # mh8[p, c, j] = 1 iff head j == 4c + p//32
    mh8 = np.zeros((128, 2, 8), np.float32)
    for c in (0, 1):
        mh8[:, c, :] = (np.arange(8)[None, :] == (4 * c + p[:, None] // 32))
    consts["mh8"] = mh8
    em8 = np.zeros((8, 2, 128), np.float32)
    for c in (0, 1):
        em8[:, c, :] = (np.arange(8)[:, None] == (4 * c + p[None, :] // 32))
    consts["em8"] = em8.astype(bf)
    consts["ident"] = np.eye(128, dtype=np.float32).astype(bf)
    return consts


def _make_in_maps(inputs):
    consts = _host_consts(inputs)
    q = np.asarray(inputs["q"], np.float32)
    k = np.asarray(inputs["k"], np.float32)
    v = np.asarray(inputs["v"], np.float32)

    in_maps = []
    for b in range(NCORES):
        m = dict(consts)
        m["q"] = np.ascontiguousarray(q[b])
        m["k"] = np.ascontiguousarray(k[b])
        m["v"] = np.ascontiguousarray(v[b])
        in_maps.append(m)
    return in_maps


def kernel(**inputs):
    nc = _get_nc()
    res = run_bass_kernel_spmd(nc, _make_in_maps(inputs), list(range(NCORES)))
    out = np.stack([np.asarray(res.results[b]["out"]) for b in range(NCORES)])
    return out.astype(np.float32)


def kernel_traced(**inputs):
    """Like kernel() but with NTFF profiling; returns (out, BassKernelResults)."""
    nc = _get_nc()
    res = run_bass_kernel_spmd(
        nc, _make_in_maps(inputs), list(range(NCORES)), trace=True
    )
    out = np.stack([np.asarray(res.results[b]["out"]) for b in range(NCORES)])
    return out.astype(np.float32), res


if __name__ == "__main__":
    rng = np.random.default_rng(0)
    ins = {
        "q": rng.standard_normal((B, L, E), np.float32),
        "k": rng.standard_normal((B, L, E), np.float32),
        "v": rng.standard_normal((B, L, E), np.float32),
        "Wq": rng.standard_normal((E, E), np.float32) / 16,
        "bq": rng.standard_normal(E).astype(np.float32) * 0.01,
        "Wk": rng.standard_normal((E, E), np.float32) / 16,
        "bk": rng.standard_normal(E).astype(np.float32) * 0.01,
        "Wv": rng.standard_normal((E, E), np.float32) / 16,
        "bv": rng.standard_normal(E).astype(np.float32) * 0.01,
        "Wm": rng.standard_normal((E, E), np.float32) / 16,
    }
    out = kernel(**ins)
    print("out", out.shape, out.dtype, np.abs(out).mean())
